# revision 1
# baseline (speedup 1.0000x reference)
"""Trainium2 Bass kernel: sliding-window GQA attention block.

Computation (matches the PyTorch/JAX reference):
    q,k,v = x @ {Wq,Wk,Wv}.T ; QK-RMSNorm ; RoPE ; GQA repeat(4x) ;
    softmax(q k^T / sqrt(D) + sliding-window bias(|i-j|<=512)) v ; @ Wo.T

Sharding (no collectives): 8 cores = 2 batches x 4 sequence chunks of 512
tokens.  Each core computes its 512 own tokens for ALL 16 heads, using a
512-token halo either side for K/V (halo K/V recomputed locally), then the
full o_proj rows for its tokens.  Outputs concatenate on host.

Layouts: projections contract over hidden, so both operands feed the PE
with hidden on partitions (host pre-transposes x and W).  Q/K are produced
directly in [head_dim, tokens] layout so attention scores^T and PV matmuls
need no on-device transposes; RMSNorm partition-dim reductions are done
with ones-vector matmuls; softmax normalization is applied after PV via a
PE-broadcast of the reciprocal denominators.  Matmul inputs use the fp32r
(reduced-mantissa fp32) PE mode: full-rate with moving dim >= 256.

Scheduling: resident tensors live in two alternating SBUF zones
(A: Wv -> x_own -> attn_out, B: Wk -> bias) so a phase's weights prefetch
on the GPSIMD/SWDGE path while the previous phase computes, instead of
stalling on the zone's previous readers.  Head dims are host-interleaved
[0,64,1,65,...] so RoPE's rotate_half is a single DVE stream-shuffle.
"""

import numpy as np


def _ensure_path():
    try:
        import concourse  # noqa: F401
    except ImportError:
        import sys
        for p in ("/opt/trn_rl_repo", "/root/.axon_site/_ro/trn_rl_repo"):
            if p not in sys.path:
                sys.path.insert(0, p)


H, KV, D = 16, 4, 128
GQ = H // KV            # 4 query heads per kv head
WIN = 512
EPS = 1e-6
B, L, HID = 2, 2048, 2048
OWN = 512               # tokens owned per core
HALO = 1536             # key/value token window per core (own +- 512)
NKT = HALO // 128       # 12 key tiles of 128
NHK = HID // 128        # 16 contraction tiles over hidden
N_CORES = 8
FMIN = np.finfo(np.float32).min
# key-tile indices whose scores need the additive mask (band edges +
# sequence-validity); tiles 4..7 are fully in-window and valid for every core
BIAS_KT = (0, 1, 2, 3, 8, 9, 10, 11)
# (key-tile, q_start, q_width): edge tiles only overlap the window for half
# the queries, so they run at half width.  kt=2 goes first: its start=True
# initialises every psum column.
KT_PLAN = [(2, 0, 512), (3, 0, 512), (4, 0, 512), (5, 0, 512),
           (6, 0, 512), (7, 0, 512), (8, 0, 512), (9, 0, 512),
           (10, 256, 256), (11, 256, 256), (0, 0, 256), (1, 0, 256)]

_CACHE = {}


def _build():
    _ensure_path()
    import concourse.mybir as mybir
    import concourse.tile as tile
    from concourse import bacc
    from contextlib import ExitStack

    F32 = mybir.dt.float32
    F32R = mybir.dt.float32r
    ACTF = mybir.ActivationFunctionType

    nc = bacc.Bacc("TRN2", target_bir_lowering=False, debug=False,
                   num_devices=N_CORES)

    xT = nc.dram_tensor("xT", [HID, HALO], F32R, kind="ExternalInput").ap()
    WqT = nc.dram_tensor("WqT", [HID, H * D], F32R, kind="ExternalInput").ap()
    WkT = nc.dram_tensor("WkT", [HID, KV * D], F32R, kind="ExternalInput").ap()
    WvT = nc.dram_tensor("WvT", [HID, KV * D], F32R, kind="ExternalInput").ap()
    WoT = nc.dram_tensor("WoT", [H * D, HID], F32R, kind="ExternalInput").ap()
    # RoPE tables, transposed to [D, tokens], norm-weights (and for q the
    # 1/sqrt(D) score scale) folded in; s-table has rotate_half sign/roll.
    cqT = nc.dram_tensor("cqT", [D, OWN], F32, kind="ExternalInput").ap()
    sqT = nc.dram_tensor("sqT", [D, OWN], F32, kind="ExternalInput").ap()
    ckT = nc.dram_tensor("ckT", [D, HALO], F32, kind="ExternalInput").ap()
    skT = nc.dram_tensor("skT", [D, HALO], F32, kind="ExternalInput").ap()
    bias8 = nc.dram_tensor("bias8", [len(BIAS_KT), 128, OWN], F32,
                           kind="ExternalInput").ap()
    out = nc.dram_tensor("out", [OWN, HID], F32, kind="ExternalOutput").ap()

    SWAP_MASK = [p ^ 1 for p in range(32)]

    with tile.TileContext(nc) as tc, ExitStack() as top:
        # ---- persistent SBUF ----
        keep = top.enter_context(tc.tile_pool(name="keep", bufs=1))
        v_sb = keep.tile([128, NKT, KV * D], F32R)      # [tok128, ktile, vf]
        kT_sb = keep.tile([128, KV, HALO], F32R)        # [d, kv, tok]
        qT_sb = keep.tile([128, H, OWN], F32R)          # [d, h, tok]
        ones32 = keep.tile([128, 1], F32)
        nc.vector.memset(ones32, 1.0)
        ones_sb = keep.tile([128, 1], F32R)
        nc.vector.tensor_copy(ones_sb, ones32)
        ones132 = keep.tile([1, 128], F32)
        nc.vector.memset(ones132, 1.0)
        ones1_sb = keep.tile([1, 128], F32R)
        nc.vector.tensor_copy(ones1_sb, ones132)

        # alternating resident zones: a phase's tensors prefetch while the
        # *other* zone's previous-phase readers drain
        zoneA = top.enter_context(tc.tile_pool(name="zoneA", bufs=1))
        zoneB = top.enter_context(tc.tile_pool(name="zoneB", bufs=1))
        xs = top.enter_context(tc.tile_pool(name="xs", bufs=4))
        ws = top.enter_context(tc.tile_pool(name="ws", bufs=5))

        def load_sliced(dst, src, n):
            for k in range(n):
                nc.gpsimd.dma_start(out=dst[:, k, :],
                                    in_=src[k * 128:(k + 1) * 128, :])

        # ================= V projection =================
        wv_sb = zoneA.tile([128, NHK, KV * D], F32R, tag="big", name="wv_sb")
        load_sliced(wv_sb, WvT, NHK)
        with ExitStack() as ph:
            ps = ph.enter_context(tc.tile_pool(name="vps", bufs=8,
                                               space="PSUM"))
            for ch in range(3):
                pv = [ps.tile([128, KV * D], F32, tag="pv", name=f"pv{t}")
                      for t in range(4)]
                for k in range(NHK):
                    xt = xs.tile([128, 512], F32R, tag="xt")
                    nc.sync.dma_start(
                        out=xt, in_=xT[k * 128:(k + 1) * 128,
                                       ch * 512:(ch + 1) * 512])
                    for tt in range(4):
                        nc.tensor.matmul(pv[tt], xt[:, tt * 128:(tt + 1) * 128],
                                         wv_sb[:, k, :],
                                         start=(k == 0), stop=(k == NHK - 1))
                for tt in range(4):
                    nc.scalar.copy(out=v_sb[:, ch * 4 + tt, :], in_=pv[tt])

        # ============ K / Q projection + RMSNorm + RoPE ============
        def norm_rope(p_feat, cT, sT, r_dst, n_tok, psn, scratch):
            """p_feat: psum [128 d, n_tok] raw head; writes r_dst (fp32r)."""
            sq = scratch.tile([128, n_tok], F32R, tag="sq")
            nc.scalar.activation(out=sq, in_=p_feat, func=ACTF.Square)
            raw = scratch.tile([128, n_tok], F32, tag="raw")
            nc.scalar.copy(out=raw, in_=p_feat)
            pss = psn.tile([1, n_tok], F32, tag="ss")
            nc.tensor.matmul(pss, ones_sb, sq, start=True, stop=True)
            ms = scratch.tile([1, n_tok], F32, tag="ms")
            nc.vector.tensor_scalar(out=ms, in0=pss, scalar1=1.0 / D,
                                    scalar2=EPS, op0=mybir.AluOpType.mult,
                                    op1=mybir.AluOpType.add)
            nc.vector.reciprocal(ms, ms)
            rs = scratch.tile([1, n_tok], F32R, tag="rs")
            nc.scalar.activation(out=rs, in_=ms, func=ACTF.Sqrt)
            prb = psn.tile([128, n_tok], F32, tag="rb")
            nc.tensor.matmul(prb, ones1_sb, rs, start=True, stop=True)
            swp = scratch.tile([128, n_tok], F32, tag="swp")
            nc.vector.stream_shuffle(out=swp, in_=raw, mask=SWAP_MASK)
            t1 = scratch.tile([128, n_tok], F32, tag="t1")
            nc.gpsimd.tensor_mul(out=t1, in0=raw, in1=cT)
            t2 = scratch.tile([128, n_tok], F32, tag="t2")
            nc.gpsimd.tensor_mul(out=t2, in0=swp, in1=sT)
            nc.gpsimd.tensor_add(out=t1, in0=t1, in1=t2)
            nc.vector.tensor_mul(out=r_dst, in0=t1, in1=prb)

        wk_sb = zoneB.tile([128, NHK, KV * D], F32R, tag="big", name="wk_sb")
        load_sliced(wk_sb, WkT, NHK)
        ck_sb = zoneB.tile([128, HALO], F32, tag="tc", name="ck_sb")
        sk_sb = zoneB.tile([128, HALO], F32, tag="ts", name="sk_sb")
        nc.gpsimd.dma_start(out=ck_sb, in_=ckT)
        nc.gpsimd.dma_start(out=sk_sb, in_=skT)

        with ExitStack() as ph:
            psp = ph.enter_context(tc.tile_pool(name="psp", bufs=6,
                                                space="PSUM"))
            psn = ph.enter_context(tc.tile_pool(name="psn", bufs=1,
                                                space="PSUM"))
            scratch = ph.enter_context(tc.tile_pool(name="scratch", bufs=2))
            for ch in range(3):
                pk = [psp.tile([128, 512], F32, tag="p", name=f"pk{t}")
                      for t in range(KV)]
                for k in range(NHK):
                    xt = xs.tile([128, 512], F32R, tag="xt")
                    nc.sync.dma_start(
                        out=xt, in_=xT[k * 128:(k + 1) * 128,
                                       ch * 512:(ch + 1) * 512])
                    for h in range(KV):
                        nc.tensor.matmul(pk[h],
                                         wk_sb[:, k, h * 128:(h + 1) * 128],
                                         xt, start=(k == 0),
                                         stop=(k == NHK - 1))
                sl = slice(ch * 512, (ch + 1) * 512)
                for h in range(KV):
                    norm_rope(pk[h], ck_sb[:, sl], sk_sb[:, sl],
                              kT_sb[:, h, sl], 512, psn, scratch)

            # ---- Q: x_own reuses zone A (Wv readers are done) ----
            xo_sb = zoneA.tile([128, NHK, OWN], F32R, tag="big", name="xo_sb")
            load_sliced(xo_sb, xT[:, 512:1024], NHK)
            cq_sb = zoneA.tile([128, OWN], F32, tag="tc", name="cq_sb")
            sq_sb = zoneA.tile([128, OWN], F32, tag="ts", name="sq_sb")
            nc.gpsimd.dma_start(out=cq_sb, in_=cqT)
            nc.gpsimd.dma_start(out=sq_sb, in_=sqT)
            for qf in range(4):
                pq = [psp.tile([128, OWN], F32, tag="p", name=f"pq{t}")
                      for t in range(4)]
                for k in range(NHK):
                    wq = ws.tile([128, 512], F32R, tag="w")
                    nc.sync.dma_start(
                        out=wq, in_=WqT[k * 128:(k + 1) * 128,
                                        qf * 512:(qf + 1) * 512])
                    for j in range(4):
                        nc.tensor.matmul(pq[j],
                                         wq[:, j * 128:(j + 1) * 128],
                                         xo_sb[:, k, :], start=(k == 0),
                                         stop=(k == NHK - 1))
                for j in range(4):
                    norm_rope(pq[j], cq_sb, sq_sb, qT_sb[:, qf * 4 + j, :],
                              OWN, psn, scratch)

        # ================= attention =================
        bias_sb = zoneB.tile([128, len(BIAS_KT), OWN], F32, tag="big",
                             name="bias_sb")
        for i in range(len(BIAS_KT)):
            nc.gpsimd.dma_start(out=bias_sb[:, i, :], in_=bias8[i])
        aoT_sb = zoneA.tile([128, H, OWN], F32R, tag="big", name="aoT_sb")
        with ExitStack() as ph:
            pss = ph.enter_context(tc.tile_pool(name="aps", bufs=3,
                                                space="PSUM"))
            pso = ph.enter_context(tc.tile_pool(name="apo", bufs=2,
                                                space="PSUM"))
            psd = ph.enter_context(tc.tile_pool(name="apd", bufs=2,
                                                space="PSUM"))
            psb = ph.enter_context(tc.tile_pool(name="apb", bufs=1,
                                                space="PSUM"))
            es = ph.enter_context(tc.tile_pool(name="aes", bufs=7))
            sc = ph.enter_context(tc.tile_pool(name="asc", bufs=2))
            for h in range(H):
                kv = h // GQ
                po = pso.tile([128, OWN], F32, tag="po")
                pd = psd.tile([1, OWN], F32, tag="pd")
                for n_kt, (kt, q0, qw) in enumerate(KT_PLAN):
                    qsl = slice(q0, q0 + qw)
                    pscr = pss.tile([128, OWN], F32, tag="ps")
                    nc.tensor.matmul(pscr[:, :qw],
                                     kT_sb[:, kv, kt * 128:(kt + 1) * 128],
                                     qT_sb[:, h, qsl], start=True, stop=True)
                    if kt in BIAS_KT:
                        idx = BIAS_KT.index(kt)
                        nc.vector.tensor_add(out=pscr[:, :qw],
                                             in0=pscr[:, :qw],
                                             in1=bias_sb[:, idx, qsl])
                    e = es.tile([128, OWN], F32R, tag="e")
                    nc.scalar.activation(out=e[:, :qw], in_=pscr[:, :qw],
                                         func=ACTF.Exp)
                    nc.tensor.matmul(po[:, qsl],
                                     v_sb[:, kt, kv * 128:(kv + 1) * 128],
                                     e[:, :qw], start=(n_kt == 0),
                                     stop=(n_kt == NKT - 1),
                                     skip_group_check=True)
                    nc.tensor.matmul(pd[:, qsl], ones_sb, e[:, :qw],
                                     start=(n_kt == 0), stop=(n_kt == NKT - 1),
                                     skip_group_check=True)
                dr32 = sc.tile([1, OWN], F32, tag="dr32")
                nc.vector.reciprocal(dr32, pd)
                dr = sc.tile([1, OWN], F32R, tag="dr")
                nc.vector.tensor_copy(dr, dr32)
                pb = psb.tile([128, OWN], F32, tag="pb")
                nc.tensor.matmul(pb, ones1_sb, dr, start=True, stop=True)
                bf = sc.tile([128, OWN], F32, tag="bf")
                nc.vector.tensor_copy(bf, pb)
                nc.vector.tensor_mul(out=aoT_sb[:, h, :], in0=po, in1=bf)

        # ================= output projection =================
        with ExitStack() as ph:
            psy = ph.enter_context(tc.tile_pool(name="ops", bufs=8,
                                                space="PSUM"))
            ys = ph.enter_context(tc.tile_pool(name="oy", bufs=4))
            for hc in range(4):
                py = [psy.tile([128, 512], F32, tag="py", name=f"py{t}")
                      for t in range(4)]
                for h in range(H):
                    wo = ws.tile([128, 512], F32R, tag="w")
                    eng = nc.sync if h % 2 == 0 else nc.scalar
                    eng.dma_start(
                        out=wo, in_=WoT[h * 128:(h + 1) * 128,
                                        hc * 512:(hc + 1) * 512])
                    for tt in range(4):
                        nc.tensor.matmul(py[tt],
                                         aoT_sb[:, h, tt * 128:(tt + 1) * 128],
                                         wo, start=(h == 0), stop=(h == H - 1))
                for tt in range(4):
                    y = ys.tile([128, 512], F32, tag="y")
                    nc.scalar.copy(out=y, in_=py[tt])
                    nc.sync.dma_start(
                        out=out[tt * 128:(tt + 1) * 128,
                                hc * 512:(hc + 1) * 512], in_=y)

    nc.compile()
    return nc


def _host_prep(x, cos, sin, Wq, Wk, Wv, Wo, q_norm_w, k_norm_w):
    """Build the 8 per-core input dicts."""
    scale = 1.0 / np.sqrt(D)
    # interleave head dims [0,64,1,65,...]: rotate_half partners end up on
    # adjacent partitions so the kernel swaps them with one stream-shuffle
    perm = np.empty(D, np.int64)
    perm[0::2] = np.arange(64)
    perm[1::2] = 64 + np.arange(64)

    def rope_tables(cos_r, sin_r, w, extra):
        # fold norm weight (and any extra scale); sign/roll for rotate_half
        c = (cos_r * w[None, :] * extra).astype(np.float32)
        w_rot = np.roll(w, -64)
        s = (sin_r * w_rot[None, :] * extra).astype(np.float32)
        s[:, :64] *= -1.0
        return (np.ascontiguousarray(c.T[perm]),
                np.ascontiguousarray(s.T[perm]))

    idx_q = (np.arange(H)[:, None] * D + perm[None, :]).ravel()
    idx_k = (np.arange(KV)[:, None] * D + perm[None, :]).ravel()
    WqT = np.ascontiguousarray(Wq.T[:, idx_q])
    WkT = np.ascontiguousarray(Wk.T[:, idx_k])
    WvT = np.ascontiguousarray(Wv.T)
    WoT = np.ascontiguousarray(Wo.T)

    in_maps = []
    for c in range(N_CORES):
        b, ch = divmod(c, 4)
        start = ch * OWN
        lo, hi = start - WIN, start + OWN + WIN
        vlo, vhi = max(lo, 0), min(hi, L)
        xh = np.zeros((HALO, HID), np.float32)
        xh[vlo - lo:vhi - lo] = x[b, vlo:vhi]
        ch_cos = np.zeros((HALO, D), np.float32)
        ch_sin = np.zeros((HALO, D), np.float32)
        ch_cos[vlo - lo:vhi - lo] = cos[vlo:vhi]
        ch_sin[vlo - lo:vhi - lo] = sin[vlo:vhi]
        ckT, skT = rope_tables(ch_cos, ch_sin, k_norm_w, 1.0)
        cqT, sqT = rope_tables(cos[start:start + OWN], sin[start:start + OWN],
                               q_norm_w, scale)
        # additive mask for the 8 edge key-tiles: [8, 128 k, OWN q]
        q_glob = start + np.arange(OWN)[None, :]
        bias = np.empty((len(BIAS_KT), 128, OWN), np.float32)
        for i, kt in enumerate(BIAS_KT):
            k_glob = (lo + kt * 128 + np.arange(128))[:, None]
            ok = (np.abs(k_glob - q_glob) <= WIN) & (k_glob >= 0) & (k_glob < L)
            bias[i] = np.where(ok, 0.0, FMIN)
        in_maps.append({
            "xT": np.ascontiguousarray(xh.T),
            "WqT": WqT, "WkT": WkT, "WvT": WvT, "WoT": WoT,
            "cqT": cqT, "sqT": sqT, "ckT": ckT, "skT": skT,
            "bias8": bias,
        })
    return in_maps


def kernel(**inputs):
    _ensure_path()
    from concourse import bass_utils

    if "nc" not in _CACHE:
        _CACHE["nc"] = _build()
    nc = _CACHE["nc"]

    in_maps = _host_prep(
        np.asarray(inputs["x"]), np.asarray(inputs["cos"]),
        np.asarray(inputs["sin"]), np.asarray(inputs["Wq"]),
        np.asarray(inputs["Wk"]), np.asarray(inputs["Wv"]),
        np.asarray(inputs["Wo"]), np.asarray(inputs["q_norm_w"]),
        np.asarray(inputs["k_norm_w"]))

    res = bass_utils.run_bass_kernel_spmd(nc, in_maps,
                                          core_ids=list(range(N_CORES)))
    out = np.empty((B, L, HID), np.float32)
    for c in range(N_CORES):
        b, ch = divmod(c, 4)
        out[b, ch * OWN:(ch + 1) * OWN] = res.results[c]["out"]
    return out



# revision 7
# speedup vs baseline: 1.2836x; 1.2836x over previous
"""Trainium2 Bass kernel: sliding-window GQA attention block.

Computation (matches the PyTorch/JAX reference):
    q,k,v = x @ {Wq,Wk,Wv}.T ; QK-RMSNorm ; RoPE ; GQA repeat(4x) ;
    softmax(q k^T / sqrt(D) + sliding-window bias(|i-j|<=512)) v ; @ Wo.T

Sharding (no collectives): 8 cores = 2 batches x 4 kv-heads.  Each core
computes K/V for its one kv head and Q/attention for its 4 query heads over
the FULL 2048-token sequence (so no K/V halo recompute), then the row-slice
of o_proj for its 512 features -> a full [L, HID] PARTIAL output; the host
sums the 4 partials per batch at gather time (o_proj row-parallel).

Everything on SBUF is bf16 (host pre-converts and pre-transposes into
partition-major [128, ...] layouts so every DMA is a single long-burst
descriptor); PSUM accumulation stays fp32.  bf16 matmuls run at 1 cycle/row
on the PE for any moving size.  Projections contract hidden on partitions;
Q/K are produced in [head_dim, tokens] layout so scores^T and PV need no
transposes; RMSNorm partition reductions are ones-vector matmuls (1/D folded
into the Square activation's scale; EPS dropped - mean(q^2) ~ 0.8 >> 1e-6);
softmax normalization is applied after PV via a PE broadcast of reciprocal
denominators.  Sliding-window edge masks are applied multiplicatively to
exp(scores) with gpsimd affine_select (no host bias tables, no DMA).

Per-(chunk, key-tile) q-ranges are tight (128-aligned), cutting score work
~22% vs uniform-width tiles.  Cross-engine normalization chains (rms -> PE
broadcast, denom -> PE broadcast) are injected into the *next* unit's matmul
stream so the PE never sits behind a DVE/Act round trip.
"""

import numpy as np


def _ensure_path():
    try:
        import concourse  # noqa: F401
    except ImportError:
        import sys
        for p in ("/opt/trn_rl_repo", "/root/.axon_site/_ro/trn_rl_repo"):
            if p not in sys.path:
                sys.path.insert(0, p)


H, KV, D = 16, 4, 128
GQ = H // KV            # 4 query heads per core (one kv head)
WIN = 512
B, L, HID = 2, 2048, 2048
NHT = HID // 128        # 16 contraction tiles over hidden
NTT = L // 128          # 16 token tiles
CH = 4                  # sequence chunks per core
CHW = L // CH           # 512 tokens per chunk
N_CORES = 8
SWAP_MASK = [p ^ 1 for p in range(32)]


def _plans():
    """Per-chunk list of (kt, q0, qw, mask_lower, mask_upper).

    kt: global 128-key tile; [q0, q0+qw) is the tight chunk-relative query
    range that has any in-window key in the tile.  mask_lower: some q-k>WIN
    pair inside the rectangle (needs select); mask_upper: some k-q>WIN.
    A full-width unmasked tile is sorted first so its PV/denominator matmul
    can initialise every PSUM column with start=True.
    """
    out = []
    for ch in range(CH):
        plans = []
        for kt in range(max(0, 4 * ch - 4), min(NTT, 4 * ch + 8)):
            k0 = kt * 128
            qa0 = max(CHW * ch, k0 - WIN)
            qa1 = min(CHW * ch + CHW, k0 + 128 + WIN)
            if qa1 <= qa0:
                continue
            mlow = (qa1 - 1) - k0 > WIN
            mup = (k0 + 127) - qa0 > WIN
            plans.append((kt, qa0 - CHW * ch, qa1 - qa0, mlow, mup))
        plans.sort(key=lambda t: (t[3] or t[4], -t[2], t[0]))
        assert not plans[0][3] and not plans[0][4] and plans[0][2] == CHW
        out.append(plans)
    return out


PLANS = _plans()

_CACHE = {}


def _build():
    _ensure_path()
    import concourse.mybir as mybir
    import concourse.tile as tile
    from concourse import bacc
    from contextlib import ExitStack

    F32 = mybir.dt.float32
    BF = mybir.dt.bfloat16
    ACTF = mybir.ActivationFunctionType
    ALU = mybir.AluOpType

    nc = bacc.Bacc("TRN2", target_bir_lowering=False, debug=False,
                   num_devices=N_CORES)

    # ---- DRAM I/O (all bf16, partition-major [128, ...]) ----
    xh = nc.dram_tensor("xh", [128, NHT * L], BF, kind="ExternalInput").ap()
    wqh = nc.dram_tensor("wqh", [128, NHT * GQ * 128], BF,
                         kind="ExternalInput").ap()
    wkh = nc.dram_tensor("wkh", [128, NHT * 128], BF,
                         kind="ExternalInput").ap()
    wvh = nc.dram_tensor("wvh", [128, NHT * 128], BF,
                         kind="ExternalInput").ap()
    woh = nc.dram_tensor("woh", [128, GQ * HID], BF,
                         kind="ExternalInput").ap()
    ckh = nc.dram_tensor("ckh", [128, L], BF, kind="ExternalInput").ap()
    skh = nc.dram_tensor("skh", [128, L], BF, kind="ExternalInput").ap()
    cqh = nc.dram_tensor("cqh", [128, L], BF, kind="ExternalInput").ap()
    sqh = nc.dram_tensor("sqh", [128, L], BF, kind="ExternalInput").ap()
    out = nc.dram_tensor("out", [L, HID], F32, kind="ExternalOutput").ap()

    with tile.TileContext(nc) as tc, ExitStack() as top:
        # ---- persistent SBUF ----
        keep = top.enter_context(tc.tile_pool(name="keep", bufs=1))
        x_sb = keep.tile([128, NHT * L], BF)          # [hid128, k*tok]
        wq_sb = keep.tile([128, NHT * GQ * 128], BF)  # [hid128, k*feat512]
        wk_sb = keep.tile([128, NHT * 128], BF)
        wv_sb = keep.tile([128, NHT * 128], BF)
        wo_sb = keep.tile([128, GQ * HID], BF)        # [feat128, f*hid]
        ck_sb = keep.tile([128, L], BF)
        sk_sb = keep.tile([128, L], BF)
        cq_sb = keep.tile([128, L], BF)
        sq_sb = keep.tile([128, L], BF)
        kT_sb = keep.tile([128, L], BF)               # [d, tok]
        qT_sb = keep.tile([128, GQ * L], BF)          # [d, h*tok]
        v_sb = keep.tile([128, NTT * 128], BF)        # [tok128, kt*dv]
        aoT_sb = keep.tile([128, GQ * L], BF)         # [dv, h*tok]

        ones32 = keep.tile([128, 128], F32)
        nc.vector.memset(ones32, 1.0)
        onesP = keep.tile([128, 1], BF)               # column of ones
        nc.vector.tensor_copy(onesP, ones32[:, 0:1])
        ones1 = keep.tile([128, 128], BF)             # all-ones (row slices)
        nc.vector.tensor_copy(ones1, ones32)

        # ---- input DMAs: x round-robin on 3 queues; weights interleaved ----
        nc.gpsimd.dma_start(out=wv_sb, in_=wvh)
        for k in range(NHT):
            eng = (nc.sync, nc.scalar, nc.gpsimd)[k % 3]
            eng.dma_start(out=x_sb[:, k * L:(k + 1) * L],
                          in_=xh[:, k * L:(k + 1) * L])
        nc.gpsimd.dma_start(out=wk_sb, in_=wkh)
        nc.gpsimd.dma_start(out=ck_sb, in_=ckh)
        nc.gpsimd.dma_start(out=sk_sb, in_=skh)
        nc.scalar.dma_start(out=wq_sb, in_=wqh)
        nc.scalar.dma_start(out=cq_sb, in_=cqh)
        nc.scalar.dma_start(out=sq_sb, in_=sqh)
        nc.sync.dma_start(out=wo_sb, in_=woh)

        # x-tile consumption order ~ DMA arrival order (3 queues round-robin)
        k_order = sorted(range(NHT), key=lambda k: (k // 3, k % 3))

        # ================= V projection =================
        # v[tok, dv] = x^T[tok, hid] @ wv[hid, dv]; 4 token-tiles per bank
        with ExitStack() as ph:
            pvp = ph.enter_context(tc.tile_pool(name="pvp", bufs=2,
                                                space="PSUM"))
            for i in range(4):
                pv = pvp.tile([128, 4, 128], F32, tag="pv")
                for n, k in enumerate(k_order):
                    for j in range(4):
                        tt = 4 * i + j
                        nc.tensor.matmul(
                            pv[:, j, :],
                            x_sb[:, k * L + tt * 128:k * L + (tt + 1) * 128],
                            wv_sb[:, k * 128:(k + 1) * 128],
                            start=(n == 0), stop=(n == NHT - 1),
                            skip_group_check=True)
                nc.scalar.copy(out=v_sb[:, i * 512:(i + 1) * 512], in_=pv)

        # ============ K / Q projection + RMSNorm + RoPE ============
        # units: 4 K chunks then 16 Q (chunk, head) pairs.  The PE parts of
        # each unit's normalization (sum-of-squares reduce, rsqrt broadcast)
        # are injected into the NEXT unit's projection matmul stream so the
        # PE never waits for the Act/DVE chain.
        units = [("k", ch, 0) for ch in range(CH)] + \
                [("q", ch, h) for ch in range(CH) for h in range(GQ)]

        with ExitStack() as ph:
            ppk = ph.enter_context(tc.tile_pool(name="ppk", bufs=2,
                                                space="PSUM"))
            prms = ph.enter_context(tc.tile_pool(name="prms", bufs=2,
                                                 space="PSUM"))
            pprb = ph.enter_context(tc.tile_pool(name="pprb", bufs=2,
                                                 space="PSUM"))
            scr = ph.enter_context(tc.tile_pool(name="scr", bufs=3))

            pend_mid = [None]
            pend_fin = [None]

            def emit_unit(kind, ch, h):
                sl = slice(ch * CHW, (ch + 1) * CHW)
                pk = ppk.tile([128, CHW], F32, tag="p", name="pk")
                for n, k in enumerate(k_order):
                    if n == 2 and pend_mid[0]:
                        pend_mid[0]()
                        pend_mid[0] = None
                    if n == NHT - 1 and pend_fin[0]:
                        pend_fin[0]()
                        pend_fin[0] = None
                    if kind == "k":
                        w = wk_sb[:, k * 128:(k + 1) * 128]
                    else:
                        w = wq_sb[:, k * GQ * 128 + h * 128:
                                  k * GQ * 128 + (h + 1) * 128]
                    nc.tensor.matmul(pk, w,
                                     x_sb[:, k * L + ch * CHW:
                                          k * L + (ch + 1) * CHW],
                                     start=(n == 0), stop=(n == NHT - 1))
                # non-PE front half: square (1/D folded into scale), raw copy,
                # rotate-half shuffle, RoPE mul/mul/add
                sq = scr.tile([128, CHW], BF, tag="sq", name="sq")
                nc.scalar.activation(out=sq, in_=pk, func=ACTF.Square,
                                     scale=float(1.0 / np.sqrt(D)))
                raw = scr.tile([128, CHW], BF, tag="raw", name="raw")
                nc.scalar.copy(out=raw, in_=pk)
                swp = scr.tile([128, CHW], BF, tag="swp", name="swp")
                nc.vector.stream_shuffle(out=swp, in_=raw, mask=SWAP_MASK)
                if kind == "k":
                    cT, sT = ck_sb[:, sl], sk_sb[:, sl]
                    dst = kT_sb[:, sl]
                else:
                    cT, sT = cq_sb[:, sl], sq_sb[:, sl]
                    dst = qT_sb[:, h * L + ch * CHW:h * L + (ch + 1) * CHW]
                t1 = scr.tile([128, CHW], BF, tag="t1", name="t1")
                nc.gpsimd.tensor_mul(out=t1, in0=raw, in1=cT)
                t2 = scr.tile([128, CHW], BF, tag="t2", name="t2")
                nc.gpsimd.tensor_mul(out=t2, in0=swp, in1=sT)
                nc.gpsimd.tensor_add(out=t1, in0=t1, in1=t2)

                rms = prms.tile([128, CHW], F32, tag="rms", name="rms")
                ms = scr.tile([128, CHW], F32, tag="ms", name="ms")
                rs = scr.tile([128, CHW], BF, tag="rs", name="rs")

                def mid():
                    # PE partition-reduce of squares, then 1/mean, sqrt
                    nc.tensor.matmul(rms[0:1, :], onesP, sq,
                                     start=True, stop=True,
                                     skip_group_check=True)
                    nc.vector.reciprocal(ms[0:1, :], rms[0:1, :])
                    nc.scalar.activation(out=rs[0:1, :],
                                         in_=ms[0:1, :], func=ACTF.Sqrt)

                def fin():
                    prb = pprb.tile([128, CHW], F32, tag="prb", name="prb")
                    nc.tensor.matmul(prb, ones1[0:1, :], rs[0:1, :],
                                     start=True, stop=True)
                    nc.vector.tensor_mul(out=dst, in0=t1, in1=prb)

                pend_mid[0] = mid
                pend_fin[0] = fin

            for kind, ch, h in units:
                emit_unit(kind, ch, h)
            pend_mid[0]()
            pend_fin[0]()

        # ================= attention + o_proj =================
        with ExitStack() as ph:
            # scores and o_proj accumulators are temporally disjoint: share
            # one 3-buffer pool so attention gets 3 score banks in flight
            psc = ph.enter_context(tc.tile_pool(name="psc", bufs=3,
                                                space="PSUM"))
            pop = ph.enter_context(tc.tile_pool(name="pop", bufs=2,
                                                space="PSUM"))
            pdp = ph.enter_context(tc.tile_pool(name="pdp", bufs=2,
                                                space="PSUM"))
            pbp = ph.enter_context(tc.tile_pool(name="pbp", bufs=1,
                                                space="PSUM"))
            es = ph.enter_context(tc.tile_pool(name="es", bufs=1))
            sc = ph.enter_context(tc.tile_pool(name="sc", bufs=1))
            ys = ph.enter_context(tc.tile_pool(name="ys", bufs=1))

            fill0 = nc.gpsimd.to_reg(0.0)

            for ch in range(CH):
                plans = PLANS[ch]
                pend_pb = [None]

                def emit_pb(po, drb, h, ch=ch):
                    # broadcast 1/denom to all partitions, then normalize
                    pb = pbp.tile([128, CHW], F32, tag="pb", name="pb")
                    nc.tensor.matmul(pb, ones1[0:1, :], drb[0:1, :],
                                     start=True, stop=True)
                    bf = sc.tile([128, CHW], BF, tag="bf", bufs=2, name="bf")
                    nc.gpsimd.tensor_copy(bf, pb)
                    nc.vector.tensor_mul(
                        out=aoT_sb[:, h * L + ch * CHW:h * L + (ch + 1) * CHW],
                        in0=po, in1=bf)

                for h in range(GQ):
                    po = pop.tile([128, CHW], F32, tag="po", name="po")
                    pd_t = pdp.tile([128, CHW], F32, tag="pd", name="pd")
                    for i, (kt, q0, qw, mlow, mup) in enumerate(plans):
                        if i == 3 and pend_pb[0]:
                            pend_pb[0]()
                            pend_pb[0] = None
                        pscr = psc.tile([128, CHW], F32, tag="ps", name="ps")
                        nc.tensor.matmul(
                            pscr[:, :qw],
                            kT_sb[:, kt * 128:(kt + 1) * 128],
                            qT_sb[:, h * L + ch * CHW + q0:
                                  h * L + ch * CHW + q0 + qw],
                            start=True, stop=True)
                        e = es.tile([128, CHW], BF, tag="e", bufs=5, name="e")
                        nc.scalar.activation(out=e[:, :qw], in_=pscr[:, :qw],
                                             func=ACTF.Exp)
                        if mlow or mup:
                            em = es.tile([128, CHW], BF, tag="em", bufs=3,
                                         name="em")
                            if mlow:  # keep where WIN + k - q >= 0
                                base = WIN + kt * 128 - ch * CHW - q0
                                cm, step = 1, -1
                            else:     # keep where WIN - k + q >= 0
                                base = WIN - kt * 128 + ch * CHW + q0
                                cm, step = -1, 1
                            nc.gpsimd.affine_select(
                                out=em[:, :qw], in_=e[:, :qw],
                                pattern=[[step, qw]],
                                compare_op=ALU.is_ge, fill=fill0,
                                base=base, channel_multiplier=cm)
                            ee = em
                        else:
                            ee = e
                        nc.tensor.matmul(po[:, q0:q0 + qw],
                                         v_sb[:, kt * 128:(kt + 1) * 128],
                                         ee[:, :qw],
                                         start=(i == 0),
                                         stop=(i == len(plans) - 1),
                                         skip_group_check=True)
                        nc.tensor.matmul(pd_t[0:1, q0:q0 + qw],
                                         onesP, ee[:, :qw],
                                         start=(i == 0),
                                         stop=(i == len(plans) - 1),
                                         skip_group_check=True)
                    # reciprocal of denominators (off the PE stream)
                    dr = sc.tile([128, CHW], F32, tag="dr", bufs=2, name="dr")
                    nc.vector.reciprocal(dr[0:1, :], pd_t[0:1, :])
                    drb = sc.tile([128, CHW], BF, tag="drb", bufs=2,
                                  name="drb")
                    nc.gpsimd.tensor_copy(drb[0:1, :], dr[0:1, :])
                    if pend_pb[0]:
                        pend_pb[0]()
                    pend_pb[0] = (lambda po=po, drb=drb, h=h:
                                  emit_pb(po, drb, h))
                pend_pb[0]()

                # ---- o_proj for this chunk: y[tok, hid] partial ----
                for tt in range(4):
                    for hc in range(4):
                        py = psc.tile([128, CHW], F32, tag="ps", name="py")
                        for f in range(GQ):
                            nc.tensor.matmul(
                                py,
                                aoT_sb[:, f * L + ch * CHW + tt * 128:
                                       f * L + ch * CHW + (tt + 1) * 128],
                                wo_sb[:, f * HID + hc * 512:
                                      f * HID + (hc + 1) * 512],
                                start=(f == 0), stop=(f == GQ - 1))
                        y = ys.tile([128, CHW], F32, tag="y", bufs=3,
                                    name="y")
                        eng = nc.vector if (tt + hc) % 2 else nc.gpsimd
                        eng.tensor_copy(y, py)
                        nc.sync.dma_start(
                            out=out[ch * CHW + tt * 128:
                                    ch * CHW + (tt + 1) * 128,
                                    hc * 512:(hc + 1) * 512],
                            in_=y)

    nc.compile()
    return nc


def _host_prep(x, cos, sin, Wq, Wk, Wv, Wo, q_norm_w, k_norm_w):
    """Build the 8 per-core input dicts (bf16, partition-major)."""
    import ml_dtypes
    BF16 = ml_dtypes.bfloat16
    scale = 1.0 / np.sqrt(D)
    # interleave head dims [0,64,1,65,...]: rotate_half partners end up on
    # adjacent partitions so the kernel swaps them with one stream-shuffle
    perm = np.empty(D, np.int64)
    perm[0::2] = np.arange(64)
    perm[1::2] = 64 + np.arange(64)

    def rope_tables(w, extra):
        c = (cos * w[None, :] * extra).astype(np.float32)
        w_rot = np.roll(w, -64)
        s = (sin * w_rot[None, :] * extra).astype(np.float32)
        s[:, :64] *= -1.0
        return (np.ascontiguousarray(c.T[perm]).astype(BF16),
                np.ascontiguousarray(s.T[perm]).astype(BF16))

    ckh, skh = rope_tables(k_norm_w, 1.0)
    cqh, sqh = rope_tables(q_norm_w, scale)

    def pmajor(a):
        # [R, C] with R = rt*128 -> [128, rt*C] (tile-of-128-rows major)
        R, C = a.shape
        return np.ascontiguousarray(
            a.reshape(R // 128, 128, C).transpose(1, 0, 2).reshape(128, -1)
        ).astype(BF16)

    WqT, WkT, WvT, WoT = Wq.T, Wk.T, Wv.T, Wo.T
    per_g = []
    for g in range(KV):
        hq = np.arange(GQ) + GQ * g
        idx_q = (hq[:, None] * D + perm[None, :]).ravel()
        idx_k = g * D + perm
        per_g.append({
            "wqh": pmajor(WqT[:, idx_q]),
            "wkh": pmajor(WkT[:, idx_k]),
            "wvh": pmajor(WvT[:, g * D:(g + 1) * D]),
            "woh": pmajor(WoT[g * GQ * D:(g + 1) * GQ * D, :]),
        })
    xb = [pmajor(x[b].T) for b in range(B)]

    in_maps = []
    for c in range(N_CORES):
        b, g = divmod(c, KV)
        m = {"xh": xb[b], "ckh": ckh, "skh": skh, "cqh": cqh, "sqh": sqh}
        m.update(per_g[g])
        in_maps.append(m)
    return in_maps


def kernel(**inputs):
    _ensure_path()
    from concourse import bass_utils

    if "nc" not in _CACHE:
        _CACHE["nc"] = _build()
    nc = _CACHE["nc"]

    in_maps = _host_prep(
        np.asarray(inputs["x"]), np.asarray(inputs["cos"]),
        np.asarray(inputs["sin"]), np.asarray(inputs["Wq"]),
        np.asarray(inputs["Wk"]), np.asarray(inputs["Wv"]),
        np.asarray(inputs["Wo"]), np.asarray(inputs["q_norm_w"]),
        np.asarray(inputs["k_norm_w"]))

    res = bass_utils.run_bass_kernel_spmd(nc, in_maps,
                                          core_ids=list(range(N_CORES)))
    out = np.zeros((B, L, HID), np.float32)
    for c in range(N_CORES):
        b = c // KV
        out[b] += res.results[c]["out"]
    return out
